# revision 43
# baseline (speedup 1.0000x reference)
"""Trainium2 Bass kernel for nn_MixerGroupedTiedAttention.

Sharding: 8 cores = (batch B=2) x (kv-group G=4). Each core handles one
batch element and one group of 4 q-heads + their shared kv-head:
  - qkv / gate projections: tensor-parallel column slices of W_qkv / W_g
  - k_rope (head-tied) replicated (folded into the per-core W slab)
  - sliding-window attention (W=1024) computed block-sparse over 128x128
    token tiles.

Design notes:
  - x is transposed to d-major on the HOST and shipped per token-tile,
    so the PE transposes + evac copies of a device-side transpose
    pipeline vanish.
  - Projections run in f32r (full-rate fp32; every PSUM dst region is
    >=256 cols, incl. a 64-col pad after krope). f32r matmuls are
    self-loading — 16-bit matmuls each pay an extra standalone
    Ldweights SEQ slot, which is why only the small attention matmuls
    use fp16/bf16 inputs (q/k fp16, probs/V bf16; validated ~3.6e-3
    rel err vs the 2e-2 gate).
  - Single fused pass per 128-token tile t: projections (tokens on PSUM
    partitions) -> rmsnorm/rope/scales -> then attention ROW t-1:
    per 128x128 key block one score matmul computes ALL 4 heads (the
    4-head q strip is the moving operand), exp'd into a per-block
    [j, (h,i)] bf16 strip that dies within the iteration; y accumulates
    per head over the 9 blocks with a ones column fused into v giving
    the softmax denominator for free. ACT/DVE attention work hides
    under the next tile's projection matmuls.
"""

import numpy as np
import ml_dtypes

D_MODEL = 2048
N_HEADS = 16
N_KV = 4
D_HEAD = 128
D1 = 64
D2 = 64
WSIZE = 1024
EPS = 1e-6
ROPE_BASE = 10000.0
B = 2
T = 2048
NCORES = 8
HPC = 4  # q heads per core
NT = T // 128  # 16 token tiles
NWB = WSIZE // 128 + 1  # 9 key tiles per query tile
WCOLS = 1280  # q(512) | kv(128) | krope(64) | pad(64) | gate(512)

_BF16 = ml_dtypes.bfloat16
_built = {}


def _build_nc():
    """Build the single-core SPMD Bass program (same program all 8 cores)."""
    if "nc" in _built:
        return _built["nc"]
    import concourse.bacc as bacc
    import concourse.tile as tile
    from concourse import mybir

    # All ACT functions this kernel uses (Copy/Square/Ln/Exp) live in the
    # "natural_log_exp_and_others" table set. The table-load pass greedily
    # picks the first set containing each function, which alternates table
    # loads (~2.7us each) between sets; restrict every other set's
    # advertised membership so exactly one table set is ever loaded.
    if not getattr(bacc, "_act_tables_pinned", False):
        _orig_gat = bacc.get_activation_tables
        _mine = {
            mybir.ActivationFunctionType.Copy,
            mybir.ActivationFunctionType.Identity,
            mybir.ActivationFunctionType.Square,
            mybir.ActivationFunctionType.Ln,
            mybir.ActivationFunctionType.Exp,
        }

        def _pinned_gat(arch):
            tabs = _orig_gat(arch)
            return {
                name: (funcs if name == "natural_log_exp_and_others"
                       else funcs - _mine)
                for name, funcs in tabs.items()
            }

        bacc.get_activation_tables = _pinned_gat
        bacc._act_tables_pinned = True

    f32 = mybir.dt.float32
    f32r = mybir.dt.float32r
    f16 = mybir.dt.float16
    bf16 = mybir.dt.bfloat16
    AF = mybir.ActivationFunctionType
    OP = mybir.AluOpType

    nc = bacc.Bacc("TRN2", target_bir_lowering=False, debug=False)

    def din(name, shape, dt):
        return nc.dram_tensor(name, shape, dt, kind="ExternalInput").ap()

    xt = din("xt", [NT, 128, D_MODEL], f32r)   # d-major x, per token tile
    xt16 = din("xt16", [3, 128, D_MODEL], f16)  # fp16 copies of tiles 0-2
    wqkv = din("wqkv", [16, 128, 768], f32r)   # q|kv|krope|pad, per chunk
    wg = din("wg", [16, 128, 512], f32r)       # gate columns, per chunk
    brk = din("brk", [128, 64], f32)
    # host pre-swizzled to partition-major: [128, NT, d]
    cost = din("cost", [128, NT, 32], f32)
    sint = din("sint", [128, NT, 32], f32)
    crow = din("crow", [128, NT, HPC], f32)
    mdiag = din("mdiag", [128, 512], bf16)     # tiled x4 for fused heads
    mfar = din("mfar", [128, 512], bf16)
    ident = din("ident", [128, 128], f16)
    out = nc.dram_tensor("out", [T, 512], f32, kind="ExternalOutput").ap()

    with tile.TileContext(nc) as tc:
        with tc.tile_pool(name="persist", bufs=1) as pp:
            wq_sb = [pp.tile([128, 768], f32r, name=f"wq_{k}")
                     for k in range(16)]
            wg_sb = [pp.tile([128, 512], f32r, name=f"wg_{k}")
                     for k in range(16)]
            brk_sb = pp.tile([128, 64], f32)
            cos_sb = pp.tile([128, NT, 32], f32)
            sin_sb = pp.tile([128, NT, 32], f32)
            crow_sb = pp.tile([128, NT, HPC], f32)
            mdiag_sb = pp.tile([128, 512], bf16)
            mfar_sb = pp.tile([128, 512], bf16)
            ident_sb = pp.tile([128, 128], f16)
            eps_sb = pp.tile([128, 1], f32)
            kT_sb = pp.tile([128, T], f16)
            vaug_sb = pp.tile([128, NT, 132], bf16)
            nc.vector.memset(eps_sb[:], EPS)
            # ones column of v_aug (softmax-denominator accumulator)
            nc.vector.memset(vaug_sb[:, :, 128], 1.0)

            with tc.tile_pool(name="xt", bufs=5) as xtp, \
                 tc.tile_pool(name="qt", bufs=3) as qtp, \
                 tc.tile_pool(name="gs", bufs=4) as gsp, \
                 tc.tile_pool(name="u", bufs=19) as up, \
                 tc.tile_pool(name="aw", bufs=2) as awp, \
                 tc.tile_pool(name="qk", bufs=3) as qkp, \
                 tc.tile_pool(name="bw", bufs=3) as bwp, \
                 tc.tile_pool(name="stg", bufs=2) as stp, \
                 tc.tile_pool(name="ppq", bufs=2, space="PSUM") as ppq, \
                 tc.tile_pool(name="ppkv", bufs=1, space="PSUM") as ppkv, \
                 tc.tile_pool(name="ppg", bufs=1, space="PSUM") as ppg, \
                 tc.tile_pool(name="psS", bufs=4, space="PSUM") as psS:

                # ---- DMA schedule: fp16 x tiles 0-2 + the qkv weight
                # stream first (cold start), gate weights + f32 x after ----
                xpre = {}

                def fetch_x(t):
                    xs = xtp.tile([128, D_MODEL], f32r, name=f"x_{t}",
                                  tag="x")
                    nc.sync.dma_start(xs[:], xt[t])
                    xpre[t] = xs

                x16_scope = tc.tile_pool(name="x16", bufs=3)
                x16p = x16_scope.__enter__()
                x16_sb = []
                for t in range(3):
                    x16 = x16p.tile([128, D_MODEL], f16, name=f"x16_{t}",
                                    tag="x16")
                    x16_sb.append(x16)
                nc.sync.dma_start(x16_sb[0][:], xt16[0])
                nc.sync.dma_start(wq_sb[0][:], wqkv[0])
                nc.sync.dma_start(x16_sb[1][:], xt16[1])
                nc.sync.dma_start(x16_sb[2][:], xt16[2])
                nc.sync.dma_start(ident_sb[:], ident[:])
                nc.sync.dma_start(brk_sb[:], brk[:])
                nc.sync.dma_start(cos_sb[:], cost[:])
                nc.sync.dma_start(sin_sb[:], sint[:])
                nc.sync.dma_start(crow_sb[:], crow[:])
                for k in range(1, 16):
                    nc.sync.dma_start(wq_sb[k][:], wqkv[k])
                # fp16 -> f32 conversion on the (idle) ACT engine; these
                # xtp tiles must be allocated BEFORE the f32 x prefetches
                # so the pool rotation matches consumption order. Tile 0 is
                # converted in 4 pieces so the first cold matmul (needing
                # only chunk 0) starts as early as possible.
                for t in range(3):
                    xs = xtp.tile([128, D_MODEL], f32r, name=f"x_{t}",
                                  tag="x")
                    if t == 0:
                        for c in range(4):
                            nc.scalar.copy(xs[:, c * 512:(c + 1) * 512],
                                           x16_sb[t][:, c * 512:(c + 1) * 512])
                    else:
                        nc.scalar.copy(xs[:], x16_sb[t][:])
                    xpre[t] = xs
                x16_scope.__exit__(None, None, None)
                nc.sync.dma_start(mdiag_sb[:], mdiag[:])
                nc.sync.dma_start(mfar_sb[:], mfar[:])
                for k in range(0, 16):
                    nc.sync.dma_start(wg_sb[k][:], wg[k])
                    if k == 4:
                        fetch_x(3)
                    elif k == 9:
                        fetch_x(4)
                    elif k == 13:
                        fetch_x(5)
                    elif k == 15:
                        fetch_x(6)
                for t in range(7, NT):
                    fetch_x(t)

                qf_t = {}
                kpre_t = {}
                gs_t = {}

                def emit_proj_qkv(t):
                    """q/kv projection matmuls for tile t (PE only)."""
                    x_slab = xpre[t]
                    pq = ppq.tile([128, 512], f32, name=f"pq_{t}", tag="pq")
                    pkv = ppkv.tile([128, 256], f32, name=f"pk_{t}",
                                    tag="pk")
                    for dst, c0, c1 in ((pkv[:], 512, 768), (pq[:], 0, 512)):
                        for k in range(16):
                            nc.tensor.matmul(
                                dst, x_slab[:, k * 128:(k + 1) * 128],
                                wq_sb[k][:, c0:c1],
                                start=(k == 0), stop=(k == 15))
                    return pq, pkv

                def emit_proj_gate(t):
                    """gate projection matmuls for tile t (PE only)."""
                    x_slab = xpre.pop(t)
                    pg = ppg.tile([128, 512], f32, name=f"pg_{t}", tag="pg")
                    for k in range(16):
                        nc.tensor.matmul(
                            pg[:], x_slab[:, k * 128:(k + 1) * 128],
                            wg_sb[k][:, 0:512],
                            start=(k == 0), stop=(k == 15))
                    return pg

                def emit_chain_qkv(t, pq, pkv):
                    """rmsnorm/rope/scales for tile t (no PE ops)."""
                    # rmsnorm scales: sumsq over each 128-wide head chunk
                    # (chunks 0-3 = q heads, 4 = kv head)
                    ss = awp.tile([128, 5], f32, name=f"ss_{t}", tag="ss")
                    sq = awp.tile([128, 128], f32, name=f"sq_{t}", tag="sq")
                    for hc in range(4):
                        nc.scalar.activation(
                            sq[:], pq[:, hc * 128:(hc + 1) * 128],
                            AF.Square, accum_out=ss[:, hc:hc + 1])
                    nc.scalar.activation(sq[:], pkv[:, 0:128], AF.Square,
                                         accum_out=ss[:, 4:5])
                    # r = rsqrt(mean+eps) via exp(-0.5*ln(.)) — keeps every
                    # ACT function in the ln/exp table set
                    lnm = awp.tile([128, 5], f32, name=f"lnm_{t}", tag="lnm")
                    nc.scalar.activation(lnm[:], ss[:], AF.Ln,
                                         scale=1.0 / 128.0, bias=eps_sb[:])
                    r = awp.tile([128, 5], f32, name=f"r_{t}", tag="r")
                    nc.scalar.activation(r[:], lnm[:], AF.Exp, scale=-0.5)
                    rc = awp.tile([128, HPC], f32, name=f"rc_{t}", tag="rc")
                    nc.vector.tensor_mul(rc[:], r[:, 0:4], crow_sb[:, t, :])

                    # q rope batched across all 4 heads (DVE muls read PSUM;
                    # Pool does the SBUF-only add/sub), then scale+fp16 cast
                    cos_t = cos_sb[:, t, :]
                    sin_t = sin_sb[:, t, :]
                    cosb = cos_t[:, None, :].broadcast_to([128, 4, 32])
                    sinb = sin_t[:, None, :].broadcast_to([128, 4, 32])
                    qh = pq[:].rearrange("p (h d) -> p h d", h=4)
                    t1 = awp.tile([128, 4, 32], f32, name=f"t1_{t}", tag="t1")
                    t2 = awp.tile([128, 4, 32], f32, name=f"t2_{t}", tag="t2")
                    t3 = awp.tile([128, 4, 32], f32, name=f"t3_{t}", tag="t3")
                    t4 = awp.tile([128, 4, 32], f32, name=f"t4_{t}", tag="t4")
                    rp = awp.tile([128, 4, 64], f32, name=f"rp_{t}", tag="rp")
                    nc.vector.tensor_mul(t1[:], qh[:, :, 64:96], cosb)
                    nc.vector.tensor_mul(t2[:], qh[:, :, 96:128], sinb)
                    nc.gpsimd.tensor_add(rp[:, :, 0:32], t1[:], t2[:])
                    nc.vector.tensor_mul(t3[:], qh[:, :, 96:128], cosb)
                    nc.vector.tensor_mul(t4[:], qh[:, :, 64:96], sinb)
                    nc.gpsimd.tensor_sub(rp[:, :, 32:64], t3[:], t4[:])
                    qf = qkp.tile([128, 4, 128], f16, name=f"qf_{t}",
                                  tag="qf")
                    rcb = rc[:, :, None].broadcast_to([128, 4, 64])
                    nc.vector.tensor_mul(qf[:, :, 0:64], qh[:, :, 0:64], rcb)
                    nc.vector.tensor_mul(qf[:, :, 64:128], rp[:], rcb)
                    qf_t[t] = qf

                    # kv head -> v (token-major) and k tied half
                    nc.vector.tensor_scalar(
                        vaug_sb[:, t, 0:128], pkv[:, 0:128],
                        r[:, 4:5], None, OP.mult)
                    kpre = qkp.tile([128, 128], f16, name=f"kp_{t}",
                                    tag="kp")
                    nc.vector.tensor_scalar(
                        kpre[:, 0:64], pkv[:, 0:64],
                        r[:, 4:5], None, OP.mult)
                    # k_rope: bias, rope (no norm)
                    krf = awp.tile([128, 64], f32, name=f"krf_{t}", tag="krf")
                    nc.vector.tensor_add(krf[:], pkv[:, 128:192], brk_sb[:])
                    k1 = awp.tile([128, 32], f32, name=f"k1_{t}", tag="k1")
                    k2 = awp.tile([128, 32], f32, name=f"k2_{t}", tag="k2")
                    k3 = awp.tile([128, 32], f32, name=f"k3_{t}", tag="k3")
                    k4 = awp.tile([128, 32], f32, name=f"k4_{t}", tag="k4")
                    nc.vector.tensor_mul(k1[:], krf[:, 0:32], cos_t)
                    nc.vector.tensor_mul(k2[:], krf[:, 32:64], sin_t)
                    nc.gpsimd.tensor_add(kpre[:, 64:96], k1[:], k2[:])
                    nc.vector.tensor_mul(k3[:], krf[:, 32:64], cos_t)
                    nc.vector.tensor_mul(k4[:], krf[:, 0:32], sin_t)
                    nc.gpsimd.tensor_sub(kpre[:, 96:128], k3[:], k4[:])
                    kpre_t[t] = kpre

                def emit_chain_gate(t, pg):
                    # gate: silu = g / (1 + exp(-g)) — exp keeps the single
                    # ACT table set; +1 on Pool; reciprocal+mul on DVE
                    gsg = awp.tile([128, 512], f32, name=f"gsg_{t}",
                                   tag="gsg")
                    nc.scalar.activation(gsg[:], pg[:], AF.Exp,
                                         scale=-1.0)
                    gw = awp.tile([128, 512], f32, name=f"gw_{t}", tag="gw")
                    nc.gpsimd.tensor_scalar_add(gw[:], gsg[:], 1.0)
                    gwi = awp.tile([128, 512], f32, name=f"gwi_{t}",
                                   tag="gwi")
                    nc.vector.reciprocal(gwi[:], gw[:])
                    g16 = gsp.tile([128, 512], f16, name=f"gs_{t}", tag="gs")
                    nc.vector.tensor_mul(g16[:], pg[:], gwi[:])
                    gs_t[t] = g16

                ublks_t = {}

                def emit_b1(r):
                    """Attention row r: transposes, scores, exp, masks."""
                    qf = qf_t.pop(r)
                    kpre = kpre_t.pop(r)
                    tq = psS.tile([128, 640], f16, name=f"tq_{r}", tag="s")
                    for h in range(HPC):
                        nc.tensor.transpose(
                            tq[:, h * 128:(h + 1) * 128], qf[:, h, :],
                            ident_sb[:])
                    nc.tensor.transpose(tq[:, 512:640], kpre[:], ident_sb[:])
                    qT = qtp.tile([128, HPC, 128], f16, name=f"qT_{r}",
                                  tag="qT")
                    nc.scalar.copy(
                        qT[:], tq[:, 0:512].rearrange("p (h t) -> p h t",
                                                      h=4))
                    nc.scalar.copy(kT_sb[:, r * 128:(r + 1) * 128],
                                   tq[:, 512:640])

                    j0 = max(0, r - (NWB - 1))
                    nblk = r - j0 + 1
                    # scores: per key block, ONE matmul for all 4 heads
                    # (moving = the 4-head q strip); exp to a [j,(h,i)] strip.
                    # The masked blocks (diag, then the clipped far block) go
                    # FIRST so their exp+mask clears before y needs them.
                    ublks = [None] * nblk
                    if nblk >= 5:
                        order = [0, 1, 2, nblk - 1] + list(range(3, nblk - 1))
                    else:
                        order = list(range(nblk))
                    for wi in order:
                        tj = j0 + wi
                        s_ps = psS.tile([128, 512], f32,
                                        name=f"s_{r}_{wi}", tag="s")
                        nc.tensor.matmul(
                            s_ps[:], kT_sb[:, tj * 128:(tj + 1) * 128],
                            qT[:], start=True, stop=True)
                        u_t = up.tile([128, 512], bf16, name=f"u_{r}_{wi}",
                                      tag="u")
                        nc.scalar.activation(u_t[:], s_ps[:], AF.Exp)
                        if tj == r:
                            nc.vector.tensor_mul(u_t[:], u_t[:], mdiag_sb[:])
                        elif nblk == NWB and wi == 0:
                            nc.vector.tensor_mul(u_t[:], u_t[:], mfar_sb[:])
                        ublks[wi] = u_t
                    ublks_t[r] = ublks

                def emit_b2(r):
                    """Attention row r: y accumulation, normalize, gate, out."""
                    j0 = max(0, r - (NWB - 1))
                    nblk = r - j0 + 1
                    ublks = ublks_t.pop(r)
                    gsr = gs_t.pop(r)
                    stage = stp.tile([128, 512], f32, name=f"o_{r}", tag="o")
                    for h in range(HPC):
                        y_ps = psS.tile([128, 132], f32, name=f"y_{r}_{h}",
                                        tag="s")
                        for wi in range(nblk):
                            tj = j0 + wi
                            nc.tensor.matmul(
                                y_ps[:, 0:129],
                                ublks[wi][:, h * 128:(h + 1) * 128],
                                vaug_sb[:, tj, 0:129],
                                start=(wi == 0), stop=(wi == nblk - 1))
                        linv = bwp.tile([128, 1], f32, name=f"li_{r}_{h}",
                                        tag="li")
                        nc.vector.reciprocal(linv[:], y_ps[:, 128:129])
                        nc.vector.scalar_tensor_tensor(
                            stage[:, h * 128:(h + 1) * 128],
                            y_ps[:, 0:128], linv[:],
                            gsr[:, h * 128:(h + 1) * 128],
                            OP.mult, OP.mult)
                    nc.sync.dma_start(out[r * 128:(r + 1) * 128, :],
                                      stage[:])

                # ---- cold start: tiles 0-2 q/kv chunk-major with
                # trailing offsets (PE tracks the weight stream); their
                # q/kv PSUM partly borrowed from the idle psS pool ----
                cold_q = [ppq.tile([128, 512], f32, name="cq_0", tag="pq"),
                          ppq.tile([128, 512], f32, name="cq_1", tag="pq"),
                          psS.tile([128, 512], f32, name="cq_2", tag="s")]
                cold_kv = [ppkv.tile([128, 256], f32, name="ck_0", tag="pk"),
                           psS.tile([128, 256], f32, name="ck_1", tag="s"),
                           psS.tile([128, 256], f32, name="ck_2", tag="s")]
                for i in range(18):
                    for t, trail in ((0, 0), (1, 1), (2, 2)):
                        k = i - trail
                        if not (0 <= k < 16):
                            continue
                        xk = xpre[t][:, k * 128:(k + 1) * 128]
                        nc.tensor.matmul(
                            cold_kv[t][:], xk, wq_sb[k][:, 512:768],
                            start=(k == 0), stop=(k == 15))
                        nc.tensor.matmul(
                            cold_q[t][:], xk, wq_sb[k][:, 0:512],
                            start=(k == 0), stop=(k == 15))
                emit_chain_qkv(0, cold_q[0], cold_kv[0])
                emit_b1(0)
                pg0 = emit_proj_gate(0)
                emit_chain_qkv(1, cold_q[1], cold_kv[1])
                emit_chain_gate(0, pg0)
                pg1 = emit_proj_gate(1)
                emit_chain_qkv(2, cold_q[2], cold_kv[2])
                emit_chain_gate(1, pg1)
                emit_b2(0)
                pg2 = emit_proj_gate(2)
                emit_chain_gate(2, pg2)
                for t in range(3, NT - 2):
                    emit_b1(t - 2)
                    pq, pkv = emit_proj_qkv(t)
                    pg = emit_proj_gate(t)
                    emit_b2(t - 2)
                    emit_chain_qkv(t, pq, pkv)
                    emit_chain_gate(t, pg)
                # t = 14: chain(14) jumps ahead of row-12 exps on ACT
                pq, pkv = emit_proj_qkv(NT - 2)
                emit_chain_qkv(NT - 2, pq, pkv)
                emit_b1(NT - 4)
                pg = emit_proj_gate(NT - 2)
                emit_b2(NT - 4)
                emit_chain_gate(NT - 2, pg)
                # t = 15: chain(15) ahead of rows 13/14 exps; rows 13/14
                # attention fills the PE while row 15's exps drain
                emit_b1(NT - 3)
                pq, pkv = emit_proj_qkv(NT - 1)
                emit_chain_qkv(NT - 1, pq, pkv)
                emit_b2(NT - 3)
                emit_b1(NT - 2)
                pg = emit_proj_gate(NT - 1)
                emit_chain_gate(NT - 1, pg)
                emit_b2(NT - 2)
                emit_b1(NT - 1)
                emit_b2(NT - 1)

    nc.compile()
    _built["nc"] = nc
    return nc


def _host_inputs(hidden_states, W_qkv, W_rk, b_rk, softmax_scaler, W_g):
    """Per-core input dicts (host-side sharding / layout / dtype prep)."""
    inv_freq = 1.0 / (ROPE_BASE ** (np.arange(0, D2, 2, dtype=np.float32) / D2))
    tpos = np.arange(T, dtype=np.float32)
    freqs = tpos[:, None] * inv_freq[None, :]
    cost = np.cos(freqs).astype(np.float32)
    sint = np.sin(freqs).astype(np.float32)
    logpos = np.log(np.minimum(tpos + 1.0, float(WSIZE))).astype(np.float32)
    scale = logpos / np.float32(np.sqrt(D_HEAD))

    ii = np.arange(128)
    mdiag = np.tile((ii[:, None] <= ii[None, :]).astype(_BF16), (1, 4))
    mfar = np.tile((ii[:, None] >= ii[None, :]).astype(_BF16), (1, 4))
    ident = np.eye(128, dtype=np.float16)
    brk_t = np.broadcast_to(
        np.asarray(b_rk, np.float32)[None, :], (128, 64)).copy()

    xf = np.asarray(hidden_states, np.float32)
    wqkv_f = np.asarray(W_qkv, np.float32)
    wrk_f = np.asarray(W_rk, np.float32)
    wg_f = np.asarray(W_g, np.float32)
    scaler = np.asarray(softmax_scaler, np.float32)
    zpad = np.zeros((D_MODEL, 64), np.float32)

    # d-major x per batch: xt[t, p, k*128+c] = x[t*128+c, k*128+p]
    xts = []
    for b in range(B):
        a = xf[b].reshape(NT, 128, 16, 128).transpose(0, 3, 2, 1)
        xts.append(np.ascontiguousarray(a.reshape(NT, 128, D_MODEL)))

    in_maps = []
    for c in range(NCORES):
        b, g = c // N_KV, c % N_KV
        qcols = wqkv_f[:, 4 * g * 128:(4 * g + 4) * 128]
        kvcols = wqkv_f[:, (N_HEADS + g) * 128:(N_HEADS + g + 1) * 128]
        gcols = wg_f[:, 4 * g * 128:(4 * g + 4) * 128]
        wall = np.concatenate([qcols, kvcols, wrk_f, zpad], axis=1)
        crow = scale[:, None] * scaler[None, 4 * g:4 * g + 4]
        in_maps.append({
            "xt": xts[b],
            "xt16": xts[b][0:3].astype(np.float16),
            "wqkv": np.ascontiguousarray(wall.reshape(16, 128, 768)),
            "wg": np.ascontiguousarray(gcols.reshape(16, 128, 512)),
            "brk": brk_t,
            # pre-swizzle (T, d) -> (128, NT, d) partition-major
            "cost": np.ascontiguousarray(
                cost.reshape(NT, 128, 32).transpose(1, 0, 2)),
            "sint": np.ascontiguousarray(
                sint.reshape(NT, 128, 32).transpose(1, 0, 2)),
            "crow": np.ascontiguousarray(
                crow.reshape(NT, 128, HPC).transpose(1, 0, 2)).astype(
                    np.float32),
            "mdiag": mdiag,
            "mfar": mfar,
            "ident": ident,
        })
    return in_maps


def kernel(hidden_states, W_qkv, W_rk, b_rk, softmax_scaler, W_g):
    from concourse.bass_utils import run_bass_kernel_spmd

    nc = _build_nc()
    in_maps = _host_inputs(hidden_states, W_qkv, W_rk, b_rk,
                           softmax_scaler, W_g)
    res = run_bass_kernel_spmd(nc, in_maps, list(range(NCORES)))
    outf = np.empty((B, T, N_HEADS, D_HEAD), np.float32)
    for c in range(NCORES):
        b, g = c // N_KV, c % N_KV
        outf[b, :, 4 * g:4 * g + 4, :] = res.results[c]["out"].reshape(
            T, HPC, D_HEAD)
    return outf


# revision 46
# speedup vs baseline: 1.0196x; 1.0196x over previous
"""Trainium2 Bass kernel for nn_MixerGroupedTiedAttention.

Sharding: 8 cores = (batch B=2) x (kv-group G=4). Each core handles one
batch element and one group of 4 q-heads + their shared kv-head:
  - qkv / gate projections: tensor-parallel column slices of W_qkv / W_g
  - k_rope (head-tied) replicated (folded into the per-core W slab)
  - sliding-window attention (W=1024) computed block-sparse over 128x128
    token tiles.

Design notes:
  - x is transposed to d-major on the HOST and shipped per token-tile,
    so the PE transposes + evac copies of a device-side transpose
    pipeline vanish.
  - Projections run in f32r (full-rate fp32; every PSUM dst region is
    >=256 cols, incl. a 64-col pad after krope). f32r matmuls are
    self-loading — 16-bit matmuls each pay an extra standalone
    Ldweights SEQ slot, which is why only the small attention matmuls
    use fp16/bf16 inputs (q/k fp16, probs/V bf16; validated ~3.6e-3
    rel err vs the 2e-2 gate).
  - Single fused pass per 128-token tile t: projections (tokens on PSUM
    partitions) -> rmsnorm/rope/scales -> then attention ROW t-1:
    per 128x128 key block one score matmul computes ALL 4 heads (the
    4-head q strip is the moving operand), exp'd into a per-block
    [j, (h,i)] bf16 strip that dies within the iteration; y accumulates
    per head over the 9 blocks with a ones column fused into v giving
    the softmax denominator for free. ACT/DVE attention work hides
    under the next tile's projection matmuls.
"""

import numpy as np
import ml_dtypes

D_MODEL = 2048
N_HEADS = 16
N_KV = 4
D_HEAD = 128
D1 = 64
D2 = 64
WSIZE = 1024
EPS = 1e-6
ROPE_BASE = 10000.0
B = 2
T = 2048
NCORES = 8
HPC = 4  # q heads per core
NT = T // 128  # 16 token tiles
NWB = WSIZE // 128 + 1  # 9 key tiles per query tile
WCOLS = 1280  # q(512) | kv(128) | krope(64) | pad(64) | gate(512)

_BF16 = ml_dtypes.bfloat16
_built = {}


def _build_nc():
    """Build the single-core SPMD Bass program (same program all 8 cores)."""
    if "nc" in _built:
        return _built["nc"]
    import concourse.bacc as bacc
    import concourse.tile as tile
    from concourse import mybir

    # All ACT functions this kernel uses (Copy/Square/Ln/Exp) live in the
    # "natural_log_exp_and_others" table set. The table-load pass greedily
    # picks the first set containing each function, which alternates table
    # loads (~2.7us each) between sets; restrict every other set's
    # advertised membership so exactly one table set is ever loaded.
    if not getattr(bacc, "_act_tables_pinned", False):
        _orig_gat = bacc.get_activation_tables
        _mine = {
            mybir.ActivationFunctionType.Copy,
            mybir.ActivationFunctionType.Identity,
            mybir.ActivationFunctionType.Square,
            mybir.ActivationFunctionType.Ln,
            mybir.ActivationFunctionType.Exp,
        }

        def _pinned_gat(arch):
            tabs = _orig_gat(arch)
            return {
                name: (funcs if name == "natural_log_exp_and_others"
                       else funcs - _mine)
                for name, funcs in tabs.items()
            }

        bacc.get_activation_tables = _pinned_gat
        bacc._act_tables_pinned = True

    f32 = mybir.dt.float32
    f32r = mybir.dt.float32r
    f16 = mybir.dt.float16
    bf16 = mybir.dt.bfloat16
    AF = mybir.ActivationFunctionType
    OP = mybir.AluOpType

    nc = bacc.Bacc("TRN2", target_bir_lowering=False, debug=False)

    def din(name, shape, dt):
        return nc.dram_tensor(name, shape, dt, kind="ExternalInput").ap()

    xt = din("xt", [NT, 128, D_MODEL], f32r)   # d-major x, per token tile
    xt16 = din("xt16", [3, 128, D_MODEL], f16)  # fp16 copies of tiles 0-2
    wqkv = din("wqkv", [16, 128, 768], f32r)   # q|kv|krope|pad, per chunk
    wg = din("wg", [16, 128, 448], f32r)       # gate cols 64:512, per chunk
    brk = din("brk", [128, 64], f32)
    # host pre-swizzled to partition-major: [128, NT, d]
    cost = din("cost", [128, NT, 32], f32)
    sint = din("sint", [128, NT, 32], f32)
    crow = din("crow", [128, NT, HPC], f32)
    mdiag = din("mdiag", [128, 512], bf16)     # tiled x4 for fused heads
    mfar = din("mfar", [128, 512], bf16)
    ident = din("ident", [128, 128], f16)
    out = nc.dram_tensor("out", [T, 512], f32, kind="ExternalOutput").ap()

    with tile.TileContext(nc) as tc:
        with tc.tile_pool(name="persist", bufs=1) as pp:
            wq_sb = [pp.tile([128, 768], f32r, name=f"wq_{k}")
                     for k in range(16)]
            wg_sb = [pp.tile([128, 448], f32r, name=f"wg_{k}")
                     for k in range(16)]
            brk_sb = pp.tile([128, 64], f32)
            cos_sb = pp.tile([128, NT, 32], f32)
            sin_sb = pp.tile([128, NT, 32], f32)
            crow_sb = pp.tile([128, NT, HPC], f32)
            mdiag_sb = pp.tile([128, 512], bf16)
            mfar_sb = pp.tile([128, 512], bf16)
            ident_sb = pp.tile([128, 128], f16)
            eps_sb = pp.tile([128, 1], f32)
            kT_sb = pp.tile([128, T], f16)
            vaug_sb = pp.tile([128, NT, 132], bf16)
            nc.vector.memset(eps_sb[:], EPS)
            # ones column of v_aug (softmax-denominator accumulator)
            nc.vector.memset(vaug_sb[:, :, 128], 1.0)

            with tc.tile_pool(name="xt", bufs=5) as xtp, \
                 tc.tile_pool(name="qt", bufs=3) as qtp, \
                 tc.tile_pool(name="gs", bufs=4) as gsp, \
                 tc.tile_pool(name="u", bufs=19) as up, \
                 tc.tile_pool(name="aw", bufs=2) as awp, \
                 tc.tile_pool(name="qk", bufs=3) as qkp, \
                 tc.tile_pool(name="bw", bufs=3) as bwp, \
                 tc.tile_pool(name="stg", bufs=2) as stp, \
                 tc.tile_pool(name="ppq", bufs=2, space="PSUM") as ppq, \
                 tc.tile_pool(name="ppkv", bufs=1, space="PSUM") as ppkv, \
                 tc.tile_pool(name="ppg", bufs=1, space="PSUM") as ppg, \
                 tc.tile_pool(name="psS", bufs=4, space="PSUM") as psS:

                # ---- DMA schedule: fp16 x tiles 0-2 + the qkv weight
                # stream first (cold start), gate weights + f32 x after ----
                xpre = {}

                def fetch_x(t):
                    xs = xtp.tile([128, D_MODEL], f32r, name=f"x_{t}",
                                  tag="x")
                    nc.sync.dma_start(xs[:], xt[t])
                    xpre[t] = xs

                x16_scope = tc.tile_pool(name="x16", bufs=3)
                x16p = x16_scope.__enter__()
                x16_sb = []
                for t in range(3):
                    x16 = x16p.tile([128, D_MODEL], f16, name=f"x16_{t}",
                                    tag="x16")
                    x16_sb.append(x16)
                nc.sync.dma_start(x16_sb[0][:], xt16[0])
                nc.sync.dma_start(wq_sb[0][:], wqkv[0])
                nc.sync.dma_start(x16_sb[1][:], xt16[1])
                nc.sync.dma_start(x16_sb[2][:], xt16[2])
                nc.sync.dma_start(ident_sb[:], ident[:])
                nc.sync.dma_start(brk_sb[:], brk[:])
                nc.sync.dma_start(cos_sb[:], cost[:])
                nc.sync.dma_start(sin_sb[:], sint[:])
                nc.sync.dma_start(crow_sb[:], crow[:])
                for k in range(1, 16):
                    nc.sync.dma_start(wq_sb[k][:], wqkv[k])
                # fp16 -> f32 conversion on the (idle) ACT engine; these
                # xtp tiles must be allocated BEFORE the f32 x prefetches
                # so the pool rotation matches consumption order. Tile 0 is
                # converted in 4 pieces so the first cold matmul (needing
                # only chunk 0) starts as early as possible.
                for t in range(3):
                    xs = xtp.tile([128, D_MODEL], f32r, name=f"x_{t}",
                                  tag="x")
                    if t == 0:
                        for c in range(4):
                            nc.scalar.copy(xs[:, c * 512:(c + 1) * 512],
                                           x16_sb[t][:, c * 512:(c + 1) * 512])
                    else:
                        nc.scalar.copy(xs[:], x16_sb[t][:])
                    xpre[t] = xs
                x16_scope.__exit__(None, None, None)
                nc.sync.dma_start(mdiag_sb[:], mdiag[:])
                nc.sync.dma_start(mfar_sb[:], mfar[:])
                for k in range(0, 16):
                    nc.sync.dma_start(wg_sb[k][:], wg[k])
                    if k == 4:
                        fetch_x(3)
                    elif k == 9:
                        fetch_x(4)
                    elif k == 13:
                        fetch_x(5)
                    elif k == 15:
                        fetch_x(6)
                for t in range(7, NT):
                    fetch_x(t)

                qf_t = {}
                kpre_t = {}
                gs_t = {}

                def emit_proj_qkv(t):
                    """q/kv projection matmuls for tile t (PE only)."""
                    x_slab = xpre[t]
                    pq = ppq.tile([128, 512], f32, name=f"pq_{t}", tag="pq")
                    pkv = ppkv.tile([128, 256], f32, name=f"pk_{t}",
                                    tag="pk")
                    for dst, c0, c1 in ((pq[:], 0, 512), (pkv[:], 512, 768)):
                        for k in range(16):
                            nc.tensor.matmul(
                                dst, x_slab[:, k * 128:(k + 1) * 128],
                                wq_sb[k][:, c0:c1],
                                start=(k == 0), stop=(k == 15))
                    return pq, pkv

                def emit_proj_gate(t):
                    """gate projection matmuls for tile t (PE only).
                    Gate cols 0:64 ride in the kvp region (former pad);
                    this computes cols 64:512."""
                    x_slab = xpre.pop(t)
                    pg = ppg.tile([128, 448], f32, name=f"pg_{t}", tag="pg")
                    for k in range(16):
                        nc.tensor.matmul(
                            pg[:], x_slab[:, k * 128:(k + 1) * 128],
                            wg_sb[k][:, 0:448],
                            start=(k == 0), stop=(k == 15))
                    return pg

                def emit_chain_qkv(t, pq, pkv):
                    """rmsnorm/rope/scales for tile t (no PE ops)."""
                    # rmsnorm scales: sumsq over each 128-wide head chunk
                    # (chunks 0-3 = q heads, 4 = kv head)
                    ss = awp.tile([128, 5], f32, name=f"ss_{t}", tag="ss")
                    sq = awp.tile([128, 128], f32, name=f"sq_{t}", tag="sq")
                    for hc in range(4):
                        nc.scalar.activation(
                            sq[:], pq[:, hc * 128:(hc + 1) * 128],
                            AF.Square, accum_out=ss[:, hc:hc + 1])
                    nc.scalar.activation(sq[:], pkv[:, 0:128], AF.Square,
                                         accum_out=ss[:, 4:5])
                    # r = rsqrt(mean+eps) via exp(-0.5*ln(.)) — keeps every
                    # ACT function in the ln/exp table set
                    lnm = awp.tile([128, 5], f32, name=f"lnm_{t}", tag="lnm")
                    nc.scalar.activation(lnm[:], ss[:], AF.Ln,
                                         scale=1.0 / 128.0, bias=eps_sb[:])
                    r = awp.tile([128, 5], f32, name=f"r_{t}", tag="r")
                    nc.scalar.activation(r[:], lnm[:], AF.Exp, scale=-0.5)
                    rc = awp.tile([128, HPC], f32, name=f"rc_{t}", tag="rc")
                    nc.vector.tensor_mul(rc[:], r[:, 0:4], crow_sb[:, t, :])

                    # q rope batched across all 4 heads (DVE muls read PSUM;
                    # Pool does the SBUF-only add/sub), then scale+fp16 cast
                    cos_t = cos_sb[:, t, :]
                    sin_t = sin_sb[:, t, :]
                    cosb = cos_t[:, None, :].broadcast_to([128, 4, 32])
                    sinb = sin_t[:, None, :].broadcast_to([128, 4, 32])
                    qh = pq[:].rearrange("p (h d) -> p h d", h=4)
                    t1 = awp.tile([128, 4, 32], f32, name=f"t1_{t}", tag="t1")
                    t2 = awp.tile([128, 4, 32], f32, name=f"t2_{t}", tag="t2")
                    t3 = awp.tile([128, 4, 32], f32, name=f"t3_{t}", tag="t3")
                    t4 = awp.tile([128, 4, 32], f32, name=f"t4_{t}", tag="t4")
                    rp = awp.tile([128, 4, 64], f32, name=f"rp_{t}", tag="rp")
                    nc.vector.tensor_mul(t1[:], qh[:, :, 64:96], cosb)
                    nc.vector.tensor_mul(t2[:], qh[:, :, 96:128], sinb)
                    nc.gpsimd.tensor_add(rp[:, :, 0:32], t1[:], t2[:])
                    nc.vector.tensor_mul(t3[:], qh[:, :, 96:128], cosb)
                    nc.vector.tensor_mul(t4[:], qh[:, :, 64:96], sinb)
                    nc.gpsimd.tensor_sub(rp[:, :, 32:64], t3[:], t4[:])
                    qf = qkp.tile([128, 4, 128], f16, name=f"qf_{t}",
                                  tag="qf")
                    rcb = rc[:, :, None].broadcast_to([128, 4, 64])
                    nc.vector.tensor_mul(qf[:, :, 0:64], qh[:, :, 0:64], rcb)
                    nc.vector.tensor_mul(qf[:, :, 64:128], rp[:], rcb)
                    qf_t[t] = qf

                    # kv head -> v (token-major) and k tied half
                    nc.vector.tensor_scalar(
                        vaug_sb[:, t, 0:128], pkv[:, 0:128],
                        r[:, 4:5], None, OP.mult)
                    kpre = qkp.tile([128, 128], f16, name=f"kp_{t}",
                                    tag="kp")
                    nc.vector.tensor_scalar(
                        kpre[:, 0:64], pkv[:, 0:64],
                        r[:, 4:5], None, OP.mult)
                    # k_rope: bias, rope (no norm)
                    krf = awp.tile([128, 64], f32, name=f"krf_{t}", tag="krf")
                    nc.vector.tensor_add(krf[:], pkv[:, 128:192], brk_sb[:])
                    k1 = awp.tile([128, 32], f32, name=f"k1_{t}", tag="k1")
                    k2 = awp.tile([128, 32], f32, name=f"k2_{t}", tag="k2")
                    k3 = awp.tile([128, 32], f32, name=f"k3_{t}", tag="k3")
                    k4 = awp.tile([128, 32], f32, name=f"k4_{t}", tag="k4")
                    nc.vector.tensor_mul(k1[:], krf[:, 0:32], cos_t)
                    nc.vector.tensor_mul(k2[:], krf[:, 32:64], sin_t)
                    nc.gpsimd.tensor_add(kpre[:, 64:96], k1[:], k2[:])
                    nc.vector.tensor_mul(k3[:], krf[:, 32:64], cos_t)
                    nc.vector.tensor_mul(k4[:], krf[:, 0:32], sin_t)
                    nc.gpsimd.tensor_sub(kpre[:, 96:128], k3[:], k4[:])
                    kpre_t[t] = kpre

                def emit_chain_gate(t, pkv, pg):
                    # gate: silu = g / (1 + exp(-g)) — exp keeps the single
                    # ACT table set; +1 on Pool; reciprocal+mul on DVE.
                    # g cols 0:64 come from the kvp PSUM (former pad cols).
                    gsg = awp.tile([128, 512], f32, name=f"gsg_{t}",
                                   tag="gsg")
                    nc.scalar.activation(gsg[:, 0:64], pkv[:, 192:256],
                                         AF.Exp, scale=-1.0)
                    nc.scalar.activation(gsg[:, 64:512], pg[:], AF.Exp,
                                         scale=-1.0)
                    gw = awp.tile([128, 512], f32, name=f"gw_{t}", tag="gw")
                    nc.gpsimd.tensor_scalar_add(gw[:], gsg[:], 1.0)
                    gwi = awp.tile([128, 512], f32, name=f"gwi_{t}",
                                   tag="gwi")
                    nc.vector.reciprocal(gwi[:], gw[:])
                    g16 = gsp.tile([128, 512], f16, name=f"gs_{t}", tag="gs")
                    nc.vector.tensor_mul(g16[:, 0:64], pkv[:, 192:256],
                                         gwi[:, 0:64])
                    nc.vector.tensor_mul(g16[:, 64:512], pg[:],
                                         gwi[:, 64:512])
                    gs_t[t] = g16

                ublks_t = {}

                def emit_b1(r):
                    """Attention row r: transposes, scores, exp, masks."""
                    qf = qf_t.pop(r)
                    kpre = kpre_t.pop(r)
                    tq = psS.tile([128, 640], f16, name=f"tq_{r}", tag="s")
                    for h in range(HPC):
                        nc.tensor.transpose(
                            tq[:, h * 128:(h + 1) * 128], qf[:, h, :],
                            ident_sb[:])
                    nc.tensor.transpose(tq[:, 512:640], kpre[:], ident_sb[:])
                    qT = qtp.tile([128, HPC, 128], f16, name=f"qT_{r}",
                                  tag="qT")
                    nc.scalar.copy(
                        qT[:], tq[:, 0:512].rearrange("p (h t) -> p h t",
                                                      h=4))
                    nc.scalar.copy(kT_sb[:, r * 128:(r + 1) * 128],
                                   tq[:, 512:640])

                    j0 = max(0, r - (NWB - 1))
                    nblk = r - j0 + 1
                    # scores: per key block, ONE matmul for all 4 heads
                    # (moving = the 4-head q strip); exp to a [j,(h,i)] strip.
                    # The masked blocks (diag, then the clipped far block) go
                    # FIRST so their exp+mask clears before y needs them.
                    ublks = [None] * nblk
                    if nblk >= 5:
                        order = [0, 1, 2, nblk - 1] + list(range(3, nblk - 1))
                    else:
                        order = list(range(nblk))
                    for wi in order:
                        tj = j0 + wi
                        s_ps = psS.tile([128, 512], f32,
                                        name=f"s_{r}_{wi}", tag="s")
                        nc.tensor.matmul(
                            s_ps[:], kT_sb[:, tj * 128:(tj + 1) * 128],
                            qT[:], start=True, stop=True)
                        u_t = up.tile([128, 512], bf16, name=f"u_{r}_{wi}",
                                      tag="u")
                        nc.scalar.activation(u_t[:], s_ps[:], AF.Exp)
                        if tj == r:
                            nc.vector.tensor_mul(u_t[:], u_t[:], mdiag_sb[:])
                        elif nblk == NWB and wi == 0:
                            nc.vector.tensor_mul(u_t[:], u_t[:], mfar_sb[:])
                        ublks[wi] = u_t
                    ublks_t[r] = ublks

                def emit_b2(r):
                    """Attention row r: y accumulation, normalize, gate, out."""
                    j0 = max(0, r - (NWB - 1))
                    nblk = r - j0 + 1
                    ublks = ublks_t.pop(r)
                    gsr = gs_t.pop(r)
                    stage = stp.tile([128, 512], f32, name=f"o_{r}", tag="o")
                    for h in range(HPC):
                        y_ps = psS.tile([128, 132], f32, name=f"y_{r}_{h}",
                                        tag="s")
                        for wi in range(nblk):
                            tj = j0 + wi
                            nc.tensor.matmul(
                                y_ps[:, 0:129],
                                ublks[wi][:, h * 128:(h + 1) * 128],
                                vaug_sb[:, tj, 0:129],
                                start=(wi == 0), stop=(wi == nblk - 1))
                        linv = bwp.tile([128, 1], f32, name=f"li_{r}_{h}",
                                        tag="li")
                        nc.vector.reciprocal(linv[:], y_ps[:, 128:129])
                        nc.vector.scalar_tensor_tensor(
                            stage[:, h * 128:(h + 1) * 128],
                            y_ps[:, 0:128], linv[:],
                            gsr[:, h * 128:(h + 1) * 128],
                            OP.mult, OP.mult)
                    nc.sync.dma_start(out[r * 128:(r + 1) * 128, :],
                                      stage[:])

                # ---- cold start: tiles 0-2 q/kv chunk-major with
                # trailing offsets (PE tracks the weight stream); their
                # q/kv PSUM partly borrowed from the idle psS pool ----
                cold_q = [ppq.tile([128, 512], f32, name="cq_0", tag="pq"),
                          ppq.tile([128, 512], f32, name="cq_1", tag="pq"),
                          psS.tile([128, 512], f32, name="cq_2", tag="s")]
                cold_kv = [ppkv.tile([128, 256], f32, name="ck_0", tag="pk"),
                           psS.tile([128, 256], f32, name="ck_1", tag="s"),
                           psS.tile([128, 256], f32, name="ck_2", tag="s")]
                for i in range(18):
                    for t, trail in ((0, 0), (1, 1), (2, 2)):
                        k = i - trail
                        if not (0 <= k < 16):
                            continue
                        xk = xpre[t][:, k * 128:(k + 1) * 128]
                        nc.tensor.matmul(
                            cold_kv[t][:], xk, wq_sb[k][:, 512:768],
                            start=(k == 0), stop=(k == 15))
                        nc.tensor.matmul(
                            cold_q[t][:], xk, wq_sb[k][:, 0:512],
                            start=(k == 0), stop=(k == 15))
                emit_chain_qkv(0, cold_q[0], cold_kv[0])
                emit_b1(0)
                pg0 = emit_proj_gate(0)
                emit_chain_qkv(1, cold_q[1], cold_kv[1])
                emit_chain_gate(0, cold_kv[0][:], pg0)
                pg1 = emit_proj_gate(1)
                emit_chain_qkv(2, cold_q[2], cold_kv[2])
                emit_chain_gate(1, cold_kv[1][:], pg1)
                emit_b2(0)
                pg2 = emit_proj_gate(2)
                emit_chain_gate(2, cold_kv[2][:], pg2)
                for t in range(3, NT - 2):
                    emit_b1(t - 2)
                    pq, pkv = emit_proj_qkv(t)
                    pg = emit_proj_gate(t)
                    emit_b2(t - 2)
                    emit_chain_qkv(t, pq, pkv)
                    emit_chain_gate(t, pkv, pg)
                # t = 14: chain(14) jumps ahead of row-12 exps on ACT
                pq, pkv = emit_proj_qkv(NT - 2)
                emit_chain_qkv(NT - 2, pq, pkv)
                emit_b1(NT - 4)
                pg = emit_proj_gate(NT - 2)
                emit_b2(NT - 4)
                emit_chain_gate(NT - 2, pkv, pg)
                # t = 15: chain(15) ahead of rows 13/14 exps; rows 13/14
                # attention fills the PE while row 15's exps drain
                emit_b1(NT - 3)
                pq, pkv = emit_proj_qkv(NT - 1)
                emit_chain_qkv(NT - 1, pq, pkv)
                emit_b2(NT - 3)
                emit_b1(NT - 2)
                pg = emit_proj_gate(NT - 1)
                emit_chain_gate(NT - 1, pkv, pg)
                emit_b2(NT - 2)
                emit_b1(NT - 1)
                emit_b2(NT - 1)

    nc.compile()
    _built["nc"] = nc
    return nc


def _host_inputs(hidden_states, W_qkv, W_rk, b_rk, softmax_scaler, W_g):
    """Per-core input dicts (host-side sharding / layout / dtype prep)."""
    inv_freq = 1.0 / (ROPE_BASE ** (np.arange(0, D2, 2, dtype=np.float32) / D2))
    tpos = np.arange(T, dtype=np.float32)
    freqs = tpos[:, None] * inv_freq[None, :]
    cost = np.cos(freqs).astype(np.float32)
    sint = np.sin(freqs).astype(np.float32)
    logpos = np.log(np.minimum(tpos + 1.0, float(WSIZE))).astype(np.float32)
    scale = logpos / np.float32(np.sqrt(D_HEAD))

    ii = np.arange(128)
    mdiag = np.tile((ii[:, None] <= ii[None, :]).astype(_BF16), (1, 4))
    mfar = np.tile((ii[:, None] >= ii[None, :]).astype(_BF16), (1, 4))
    ident = np.eye(128, dtype=np.float16)
    brk_t = np.broadcast_to(
        np.asarray(b_rk, np.float32)[None, :], (128, 64)).copy()

    xf = np.asarray(hidden_states, np.float32)
    wqkv_f = np.asarray(W_qkv, np.float32)
    wrk_f = np.asarray(W_rk, np.float32)
    wg_f = np.asarray(W_g, np.float32)
    scaler = np.asarray(softmax_scaler, np.float32)
    zpad = np.zeros((D_MODEL, 64), np.float32)

    # d-major x per batch: xt[t, p, k*128+c] = x[t*128+c, k*128+p]
    xts = []
    for b in range(B):
        a = xf[b].reshape(NT, 128, 16, 128).transpose(0, 3, 2, 1)
        xts.append(np.ascontiguousarray(a.reshape(NT, 128, D_MODEL)))

    in_maps = []
    for c in range(NCORES):
        b, g = c // N_KV, c % N_KV
        qcols = wqkv_f[:, 4 * g * 128:(4 * g + 4) * 128]
        kvcols = wqkv_f[:, (N_HEADS + g) * 128:(N_HEADS + g + 1) * 128]
        gcols = wg_f[:, 4 * g * 128:(4 * g + 4) * 128]
        wall = np.concatenate([qcols, kvcols, wrk_f, gcols[:, 0:64]], axis=1)
        crow = scale[:, None] * scaler[None, 4 * g:4 * g + 4]
        in_maps.append({
            "xt": xts[b],
            "xt16": xts[b][0:3].astype(np.float16),
            "wqkv": np.ascontiguousarray(wall.reshape(16, 128, 768)),
            "wg": np.ascontiguousarray(
                np.ascontiguousarray(gcols[:, 64:512]).reshape(16, 128, 448)),
            "brk": brk_t,
            # pre-swizzle (T, d) -> (128, NT, d) partition-major
            "cost": np.ascontiguousarray(
                cost.reshape(NT, 128, 32).transpose(1, 0, 2)),
            "sint": np.ascontiguousarray(
                sint.reshape(NT, 128, 32).transpose(1, 0, 2)),
            "crow": np.ascontiguousarray(
                crow.reshape(NT, 128, HPC).transpose(1, 0, 2)).astype(
                    np.float32),
            "mdiag": mdiag,
            "mfar": mfar,
            "ident": ident,
        })
    return in_maps


def kernel(hidden_states, W_qkv, W_rk, b_rk, softmax_scaler, W_g):
    from concourse.bass_utils import run_bass_kernel_spmd

    nc = _build_nc()
    in_maps = _host_inputs(hidden_states, W_qkv, W_rk, b_rk,
                           softmax_scaler, W_g)
    res = run_bass_kernel_spmd(nc, in_maps, list(range(NCORES)))
    outf = np.empty((B, T, N_HEADS, D_HEAD), np.float32)
    for c in range(NCORES):
        b, g = c // N_KV, c % N_KV
        outf[b, :, 4 * g:4 * g + 4, :] = res.results[c]["out"].reshape(
            T, HPC, D_HEAD)
    return outf
